# revision 16
# baseline (speedup 1.0000x reference)
"""BatchChildSumTreeLSTM Trainium2 kernel.

Forest of T complete B-ary trees, level-major node layout, processed
leaves-first.  Sharded across 8 NeuronCores by tree (2 trees / core);
every level's per-core slice is contiguous, so the whole recursion is
device-local.

Per-core layout is feature-major: activations live in SBUF as
[128 features (partitions), n nodes (free)].  Matmuls are
out[H, n] = W.T.T @ rhs with pre-transposed weights as lhsT, so matmul
outputs are directly consumable as later matmul inputs and PSUM
accumulates the x-projection with the h-recurrence.  The host feeds:
  xT  [128, ntot]  embeds transposed (per-core shard)
  xb  [128, ntot]  parent embedding broadcast to each child column
                   (level-0 columns unused) -> f-gate is 2 matmuls.
Gates are evaluated by ScalarE straight out of PSUM with per-partition
bias APs; VectorE does the elementwise muls and the fc child-sum
(pairwise tree); GpSimd does the h child-sum.  Level 6 h/c spill to
DRAM (too big for SBUF together with the level-7 streaming chunks);
levels <= 5 stay resident.
"""

import sys

if "/opt/trn_rl_repo" not in sys.path:
    sys.path.insert(0, "/opt/trn_rl_repo")

import numpy as np

P = 128          # feature dim == partitions
BR = 4           # branching factor
NLBL = 5

F32 = None       # filled lazily (mybir import is heavy)

_NC_CACHE = {}


def _levels(tpc, depth):
    n = [tpc * BR**l for l in range(depth)]
    off = [0]
    for c in n:
        off.append(off[-1] + c)
    return n, off, off[-1]


def _legalize_waits(nc, max_waits=1):
    """This walrus build accepts at most one sync-wait command per
    instruction (any type).  Hoist excess waits onto same-engine NoOps
    inserted right before the instruction; engine program order makes
    this exactly equivalent."""
    import concourse.mybir as mybir

    n_nops = 0
    for fn in nc.m.functions:
        for blk in fn.blocks:
            new_insts = []
            for inst in blk.instructions:
                si = getattr(inst, "sync_info", None)
                if si is not None and si.on_wait and len(si.on_wait) > max_waits:
                    waits = list(si.on_wait)
                    hoist, keep = waits[:-max_waits], waits[-max_waits:]
                    eng = getattr(inst, "engine", None)
                    for j, w in enumerate(hoist):
                        nop = mybir.InstNoOp(
                            name=f"{inst.name}-wn{j}",
                            engine=eng,
                            bass_nofuse=True,
                        )
                        nop.sync_info = mybir.SyncInfo(on_wait=[w],
                                                       on_update=[])
                        new_insts.append(nop)
                        n_nops += 1
                    inst.sync_info = mybir.SyncInfo(
                        on_wait=keep, on_update=list(si.on_update))
                new_insts.append(inst)
            blk.instructions = new_insts
    return n_nops


def build_nc(tpc=2, depth=8, ch_leaf=2048, resident_max=2048, nlbl=NLBL,
             legalize=True):
    """Build the per-core Bass/Tile program (identical on all cores)."""
    import concourse.bass as bass
    import concourse.mybir as mybir
    import concourse.tile as tile

    f32 = mybir.dt.float32
    bf16 = mybir.dt.bfloat16
    AF = mybir.ActivationFunctionType
    ADD = mybir.AluOpType.add

    n, off, ntot = _levels(tpc, depth)
    lleaf = depth - 1
    nleaf = n[lleaf]
    ch_leaf = min(ch_leaf, nleaf)
    assert nleaf % ch_leaf == 0

    nc = bass.Bass()

    xT = nc.dram_tensor("xT", [P, ntot], bf16, kind="ExternalInput")
    xb = nc.dram_tensor("xb", [P, ntot], bf16, kind="ExternalInput")
    wnames = ["ix", "ih", "fx", "fh", "ox", "oh", "ux", "uh"]
    wall = nc.dram_tensor("Wall", [P, len(wnames), P], bf16,
                          kind="ExternalInput")
    bias4 = nc.dram_tensor("bias4", [P, 4], f32, kind="ExternalInput")
    woutT = nc.dram_tensor("WoutT", [P, nlbl], bf16, kind="ExternalInput")
    bout = nc.dram_tensor("bout2", [tpc, nlbl], f32, kind="ExternalInput")
    out = nc.dram_tensor("out", [tpc, nlbl], f32, kind="ExternalOutput")

    assert depth >= 4

    SIG, TANH = AF.Sigmoid, AF.Tanh
    BIDX = {"i": 0, "f": 1, "o": 2, "u": 3}

    with tile.TileContext(nc) as tc:
        import contextlib
        with contextlib.ExitStack() as ctx:
            wp = ctx.enter_context(tc.tile_pool(name="wp", bufs=1))
            stream = ctx.enter_context(tc.tile_pool(name="stream", bufs=2))
            tmp = ctx.enter_context(tc.tile_pool(name="tmp", bufs=2))
            acc = ctx.enter_context(tc.tile_pool(name="acc", bufs=2))
            chk = ctx.enter_context(tc.tile_pool(name="chk", bufs=2))
            hcp = ctx.enter_context(tc.tile_pool(name="hcp", bufs=1))
            psum = ctx.enter_context(
                tc.tile_pool(name="psum", bufs=2, space="PSUM"))

            GMAX = 2048  # max columns per gate/f group (4 PSUM banks)

            # ---- load constants ----
            wall_sb = wp.tile([P, len(wnames), P], bf16, name="wall_sb",
                              tag="wall_sb")
            nc.sync.dma_start(out=wall_sb, in_=wall[:])
            wsb = {nm: wall_sb[:, j, :] for j, nm in enumerate(wnames)}
            bias_sb = wp.tile([P, 4], f32, name="bias_sb", tag="bias_sb")
            nc.sync.dma_start(out=bias_sb, in_=bias4[:])
            woutT_sb = wp.tile([P, nlbl], bf16, name="woutT_sb", tag="woutT_sb")
            nc.sync.dma_start(out=woutT_sb, in_=woutT[:])
            bout_sb = wp.tile([tpc, nlbl], f32, name="bout_sb", tag="bout_sb")
            nc.sync.dma_start(out=bout_sb, in_=bout[:])

            def bias_ap(g):
                i = BIDX[g]
                return bias_sb[:, i:i + 1]

            # persistent h/c for resident levels (leaves and level
            # depth-2 are streamed/consumed in flight, never stored whole)
            hres, cres = {}, {}
            for l in range(depth - 2):
                hres[l] = hcp.tile([P, n[l]], bf16, name=f"h{l}_sb",
                                   tag=f"h{l}_sb")
                cres[l] = hcp.tile([P, n[l]], f32, name=f"c{l}_sb",
                                   tag=f"c{l}_sb")

            def matmul_group(ps, w0, rhs0, w1=None, rhs1=None, G=GMAX):
                """ps[:, :G] = w0.T@rhs0 (+ w1.T@rhs1).  512-col banks."""
                nb = (G + 511) // 512
                for b in range(nb):
                    s = b * 512
                    e = min(s + 512, G)
                    nc.tensor.matmul(ps[:, s:e], wsb[w0], rhs0[:, s:e],
                                     start=True, stop=(w1 is None))
                if w1 is not None:
                    for b in range(nb):
                        s = b * 512
                        e = min(s + 512, G)
                        nc.tensor.matmul(ps[:, s:e], wsb[w1], rhs1[:, s:e],
                                         start=False, stop=True)

            def gate_pass(xt_ap, hs_ap, fcs_ap, h_out, c_out, G):
                """Compute i,u,o gates + c,h for G parent columns."""
                leaf = hs_ap is None
                g_sb = {}
                for gname, wx, wh, func in (("i", "ix", "ih", SIG),
                                            ("u", "ux", "uh", TANH),
                                            ("o", "ox", "oh", SIG)):
                    ps = psum.tile([P, GMAX], f32, name=f"ps_{gname}",
                                   tag="ps")
                    if leaf:
                        matmul_group(ps, wx, xt_ap, G=G)
                    else:
                        matmul_group(ps, wx, xt_ap, wh, hs_ap, G=G)
                    gdt = bf16 if gname == "o" else f32
                    g = tmp.tile([P, G], gdt, name=f"g_{gname}",
                                 tag=f"g_{gname}")
                    nc.scalar.activation(out=g, in_=ps[:, :G], func=func,
                                         bias=bias_ap(gname), scale=1.0)
                    g_sb[gname] = g
                nc.vector.tensor_mul(c_out, g_sb["i"], g_sb["u"])
                if fcs_ap is not None:
                    nc.vector.tensor_add(c_out, c_out, fcs_ap)
                tt = tmp.tile([P, G], bf16, name="tt", tag="tt")
                nc.scalar.activation(out=tt, in_=c_out, func=TANH,
                                     bias=0.0, scale=1.0)
                nc.vector.tensor_mul(h_out, g_sb["o"], tt)

            def f_pass(xb_ap, hch_ap, cch_ap, fcs_out, hs_out, S):
                """f gates for S child cols; reduce fc and h by groups of 4."""
                ps = psum.tile([P, GMAX], f32, name="ps_f", tag="ps")
                matmul_group(ps, "fx", xb_ap, "fh", hch_ap, G=S)
                f = tmp.tile([P, S], f32, name="g_f", tag="g_f")
                nc.scalar.activation(out=f, in_=ps[:, :S], func=SIG,
                                     bias=bias_ap("f"), scale=1.0)
                fc = tmp.tile([P, S], f32, name="fc", tag="fc")
                nc.vector.tensor_mul(fc, f, cch_ap)
                # pairwise reduce x4 -> fcs_out  (GpSimd; VectorE is busier)
                fcv = fc.rearrange("p (a two) -> p a two", two=2)
                rt = tmp.tile([P, S // 2], f32, name="rt", tag="rt",
                              bufs=1)
                nc.gpsimd.tensor_tensor(rt, fcv[:, :, 0], fcv[:, :, 1], ADD)
                rtv = rt.rearrange("p (a two) -> p a two", two=2)
                nc.gpsimd.tensor_tensor(fcs_out, rtv[:, :, 0], rtv[:, :, 1],
                                        ADD)
                # pairwise reduce h x4 -> hs_out  (GpSimd)
                hv = hch_ap.rearrange("p (a two) -> p a two", two=2)
                rt2 = tmp.tile([P, S // 2], bf16, name="rt2", tag="rt2",
                               bufs=1)
                nc.gpsimd.tensor_tensor(rt2, hv[:, :, 0], hv[:, :, 1], ADD)
                rt2v = rt2.rearrange("p (a two) -> p a two", two=2)
                nc.gpsimd.tensor_tensor(hs_out, rt2v[:, :, 0], rt2v[:, :, 1],
                                        ADD)

            def load_stream(tag, src, a, b, dt=bf16):
                t = stream.tile([P, b - a], dt, name=tag, tag=tag)
                nc.sync.dma_start(out=t, in_=src[:, a:b])
                return t

            # ================= leaves fused with level depth-2 ============
            l6 = depth - 2
            GQ = min(GMAX, n[l6])          # parents per level-l6 gate group
            cpq = max(1, (GQ * BR) // ch_leaf)   # leaf chunks per gate group
            assert GQ * BR == cpq * ch_leaf
            nchunks = nleaf // ch_leaf
            fcs = hs = None
            pending = None
            l5 = depth - 3
            q5 = GQ // BR
            # level depth-3 child-sum accumulators, filled group by group
            fcs5 = acc.tile([P, n[l5]], f32, name="fcs5", tag="fcs5", bufs=1)
            hs5 = acc.tile([P, n[l5]], bf16, name="hs5", tag="hs5", bufs=1)

            def emit_l6_gates(xt6, hs_p, fcs_p, p0):
                h6o = chk.tile([P, GQ], bf16, name="h6o", tag="h6o")
                c6o = chk.tile([P, GQ], f32, name="c6o", tag="c6o")
                gate_pass(xt6, hs_p, fcs_p, h6o, c6o, GQ)
                return (h6o, c6o, p0)

            def emit_l5_f(h6o, c6o, p0):
                # f-pass of level depth-3 over the group produced above
                xbt5 = load_stream("xb", xb, off[l6] + p0, off[l6] + p0 + GQ)
                a5 = p0 // BR
                f_pass(xbt5, h6o, c6o, fcs5[:, a5:a5 + q5],
                       hs5[:, a5:a5 + q5], GQ)

            pending_l5f = None
            for k in range(nchunks):
                xt7 = load_stream("xt", xT, off[lleaf] + k * ch_leaf,
                                  off[lleaf] + (k + 1) * ch_leaf)
                h7k = stream.tile([P, ch_leaf], bf16, name="hch", tag="hch")
                c7k = stream.tile([P, ch_leaf], f32, name="cch", tag="cch")
                gate_pass(xt7, None, None, h7k, c7k, ch_leaf)
                # deferred work from previous groups: ACT chews the leaf
                # sigmoids above while PE runs these matmuls
                if pending_l5f is not None:
                    emit_l5_f(*pending_l5f)
                    pending_l5f = None
                if pending is not None:
                    pending_l5f = emit_l6_gates(*pending)
                    pending = None
                # f-pass of level l6 over this chunk's children
                xbt = load_stream("xb", xb, off[lleaf] + k * ch_leaf,
                                  off[lleaf] + (k + 1) * ch_leaf)
                jq = k % cpq
                if jq == 0:
                    fcs = acc.tile([P, GQ], f32, name="fcs", tag="fcs")
                    hs = acc.tile([P, GQ], bf16, name="hs", tag="hs")
                q = ch_leaf // BR
                f_pass(xbt, h7k, c7k, fcs[:, jq * q:(jq + 1) * q],
                       hs[:, jq * q:(jq + 1) * q], ch_leaf)
                if jq == cpq - 1:
                    gq_i = k // cpq
                    p0 = gq_i * GQ
                    xt6 = load_stream("xt", xT, off[l6] + p0,
                                      off[l6] + p0 + GQ)
                    pending = (xt6, hs, fcs, p0)
            if pending_l5f is not None:
                emit_l5_f(*pending_l5f)
                pending_l5f = None
            if pending is not None:
                pending_l5f = emit_l6_gates(*pending)
                pending = None
            if pending_l5f is not None:
                emit_l5_f(*pending_l5f)
                pending_l5f = None

            # gates of level depth-3 (its child-sums are complete now)
            xt5 = load_stream("xt", xT, off[l5], off[l5] + n[l5])
            gate_pass(xt5, hs5, fcs5, hres[l5], cres[l5], n[l5])

            # ================= levels depth-4 .. 0 ========================
            for l in range(depth - 4, -1, -1):
                npar, nch = n[l], n[l + 1]
                S = min(GMAX, nch)
                ngr = nch // S
                fcs = acc.tile([P, npar], f32, name="fcs", tag="fcs")
                hs = acc.tile([P, npar], bf16, name="hs", tag="hs")
                for g in range(ngr):
                    a = g * S
                    hch = hres[l + 1][:, a:a + S]
                    cch = cres[l + 1][:, a:a + S]
                    xbt = load_stream("xb", xb, off[l + 1] + a,
                                      off[l + 1] + a + S)
                    q = S // BR
                    f_pass(xbt, hch, cch, fcs[:, g * q:(g + 1) * q],
                           hs[:, g * q:(g + 1) * q], S)
                xt = load_stream("xt", xT, off[l], off[l] + npar)
                gate_pass(xt, hs, fcs, hres[l], cres[l], npar)

            # ================= output head ================================
            ps = psum.tile([P, GMAX], f32, name="ps_out", tag="ps")
            nc.tensor.matmul(ps[:tpc, :nlbl], hres[0], woutT_sb,
                             start=True, stop=True)
            out_sb = tmp.tile([tpc, nlbl], f32, name="out_sb", tag="out_sb")
            nc.vector.tensor_add(out_sb, ps[:tpc, :nlbl], bout_sb)
            nc.sync.dma_start(out=out[:], in_=out_sb)

    if legalize:
        _legalize_waits(nc)
    return nc


def _prep_core_inputs(embeds, weights, tpc, depth, n_cores):
    """Host-side shard + transpose + parent-broadcast per core."""
    n, off, ntot = _levels(tpc, depth)
    T = tpc * n_cores
    counts = [T * BR**l for l in range(depth)]
    offsets = [0]
    for c in counts:
        offsets.append(offsets[-1] + c)

    common = dict(weights)
    in_maps = []
    for d in range(n_cores):
        import ml_dtypes
        bf16 = ml_dtypes.bfloat16
        shard = np.concatenate(
            [embeds[offsets[l] + tpc * d * BR**l:
                    offsets[l] + tpc * (d + 1) * BR**l] for l in range(depth)],
            axis=0)
        xT = np.ascontiguousarray(shard.T.astype(bf16))   # [P, ntot]
        xbm = np.zeros_like(xT)
        for l in range(1, depth):
            xbm[:, off[l]:off[l + 1]] = np.repeat(
                xT[:, off[l - 1]:off[l]], BR, axis=1)
        m = {"xT": xT, "xb": xbm}
        m.update(common)
        in_maps.append(m)
    return in_maps


def _prep_weights(Wix, bix, Wih, Wfx, bfx, Wfh, Wox, box, Woh, Wux, bux, Wuh,
                  Wout, bout, tpc):
    import ml_dtypes
    f = np.float32
    bf = ml_dtypes.bfloat16
    # order must match build_nc's wnames: ix, ih, fx, fh, ox, oh, ux, uh
    wall = np.stack([Wix.T, Wih.T, Wfx.T, Wfh.T, Wox.T, Woh.T, Wux.T, Wuh.T],
                    axis=1)   # [128 (in-feat), 8, 128 (out-feat)]
    w = {
        "Wall": np.ascontiguousarray(wall, dtype=bf),
        "bias4": np.ascontiguousarray(
            np.stack([bix, bfx, box, bux], axis=1), dtype=f),
        "WoutT": np.ascontiguousarray(Wout.T, dtype=bf),
        "bout2": np.ascontiguousarray(np.tile(bout, (tpc, 1)), dtype=f),
    }
    return w


def _ensure_ntff_hook():
    """The RL container's antenv lacks axon_hooks; install a shim and
    register the ctypes NTFF profiler so trace=True works."""
    import types

    try:
        from antenv.axon_hooks import get_axon_ntff_profile_hook  # noqa
        return
    except ImportError:
        pass
    mod = types.ModuleType("antenv.axon_hooks")
    _h = [None]
    mod.set_axon_ntff_profile_hook = lambda h: _h.__setitem__(0, h)
    mod.get_axon_ntff_profile_hook = lambda: _h[0]
    sys.modules["antenv.axon_hooks"] = mod
    import antenv
    antenv.axon_hooks = mod
    try:
        from trn_agent_boot.trn_boot import _ntff_profile_via_ctypes
        h = _ntff_profile_via_ctypes("/opt/axon/libaxon_pjrt.so")
        if h is not None:
            mod.set_axon_ntff_profile_hook(h)
    except Exception:
        pass


def kernel(embeds, Wix, bix, Wih, Wfx, bfx, Wfh, Wox, box, Woh, Wux, bux, Wuh,
           Wout, bout, _trace=False):
    from concourse import bass_utils
    from concourse.bass_utils import run_bass_kernel_spmd

    if _trace:
        _ensure_ntff_hook()
        bass_utils.upload_artifacts = lambda d: d  # no S3 in this container

    n_cores = 8
    depth = 8
    T = 16
    tpc = T // n_cores

    embeds = np.asarray(embeds, dtype=np.float32)
    weights = _prep_weights(
        np.asarray(Wix), np.asarray(bix), np.asarray(Wih), np.asarray(Wfx),
        np.asarray(bfx), np.asarray(Wfh), np.asarray(Wox), np.asarray(box),
        np.asarray(Woh), np.asarray(Wux), np.asarray(bux), np.asarray(Wuh),
        np.asarray(Wout), np.asarray(bout), tpc)
    in_maps = _prep_core_inputs(embeds, weights, tpc, depth, n_cores)

    key = (tpc, depth)
    if key not in _NC_CACHE:
        _NC_CACHE[key] = build_nc(tpc=tpc, depth=depth)
    nc = _NC_CACHE[key]

    res = run_bass_kernel_spmd(nc, in_maps, core_ids=list(range(n_cores)),
                               trace=_trace)
    outs = np.concatenate([r["out"] for r in res.results], axis=0)
    if _trace:
        kernel.last_results = res
    return outs.astype(np.float32)


kernel.last_results = None


# revision 17
# speedup vs baseline: 1.0697x; 1.0697x over previous
"""BatchChildSumTreeLSTM Trainium2 kernel.

Forest of T complete B-ary trees, level-major node layout, processed
leaves-first.  Sharded across 8 NeuronCores by tree (2 trees / core);
every level's per-core slice is contiguous, so the whole recursion is
device-local.

Per-core layout is feature-major: activations live in SBUF as
[128 features (partitions), n nodes (free)].  Matmuls are
out[H, n] = W.T.T @ rhs with pre-transposed weights as lhsT, so matmul
outputs are directly consumable as later matmul inputs and PSUM
accumulates the x-projection with the h-recurrence.  The host feeds:
  xT  [128, ntot]  embeds transposed (per-core shard)
  xb  [128, ntot]  parent embedding broadcast to each child column
                   (level-0 columns unused) -> f-gate is 2 matmuls.
Gates are evaluated by ScalarE straight out of PSUM with per-partition
bias APs; VectorE does the elementwise muls and the fc child-sum
(pairwise tree); GpSimd does the h child-sum.  Level 6 h/c spill to
DRAM (too big for SBUF together with the level-7 streaming chunks);
levels <= 5 stay resident.
"""

import sys

if "/opt/trn_rl_repo" not in sys.path:
    sys.path.insert(0, "/opt/trn_rl_repo")

import numpy as np

P = 128          # feature dim == partitions
BR = 4           # branching factor
NLBL = 5

F32 = None       # filled lazily (mybir import is heavy)

_NC_CACHE = {}


def _levels(tpc, depth):
    n = [tpc * BR**l for l in range(depth)]
    off = [0]
    for c in n:
        off.append(off[-1] + c)
    return n, off, off[-1]


def _legalize_waits(nc, max_waits=1):
    """This walrus build accepts at most one sync-wait command per
    instruction (any type).  Hoist excess waits onto same-engine NoOps
    inserted right before the instruction; engine program order makes
    this exactly equivalent."""
    import concourse.mybir as mybir

    n_nops = 0
    for fn in nc.m.functions:
        for blk in fn.blocks:
            new_insts = []
            for inst in blk.instructions:
                si = getattr(inst, "sync_info", None)
                if si is not None and si.on_wait and len(si.on_wait) > max_waits:
                    waits = list(si.on_wait)
                    hoist, keep = waits[:-max_waits], waits[-max_waits:]
                    eng = getattr(inst, "engine", None)
                    for j, w in enumerate(hoist):
                        nop = mybir.InstNoOp(
                            name=f"{inst.name}-wn{j}",
                            engine=eng,
                            bass_nofuse=True,
                        )
                        nop.sync_info = mybir.SyncInfo(on_wait=[w],
                                                       on_update=[])
                        new_insts.append(nop)
                        n_nops += 1
                    inst.sync_info = mybir.SyncInfo(
                        on_wait=keep, on_update=list(si.on_update))
                new_insts.append(inst)
            blk.instructions = new_insts
    return n_nops


def build_nc(tpc=2, depth=8, ch_leaf=2048, resident_max=2048, nlbl=NLBL,
             legalize=True):
    """Build the per-core Bass/Tile program (identical on all cores)."""
    import concourse.bass as bass
    import concourse.mybir as mybir
    import concourse.tile as tile

    f32 = mybir.dt.float32
    bf16 = mybir.dt.bfloat16
    AF = mybir.ActivationFunctionType
    ADD = mybir.AluOpType.add

    n, off, ntot = _levels(tpc, depth)
    lleaf = depth - 1
    nleaf = n[lleaf]
    ch_leaf = min(ch_leaf, nleaf)
    assert nleaf % ch_leaf == 0

    nc = bass.Bass()

    xT = nc.dram_tensor("xT", [P, ntot], bf16, kind="ExternalInput")
    xb = nc.dram_tensor("xb", [P, ntot], bf16, kind="ExternalInput")
    wnames = ["ix", "ih", "fx", "fh", "ox", "oh", "ux", "uh"]
    wall = nc.dram_tensor("Wall", [P, len(wnames), P], bf16,
                          kind="ExternalInput")
    bias4 = nc.dram_tensor("bias4", [P, 4], f32, kind="ExternalInput")
    woutT = nc.dram_tensor("WoutT", [P, nlbl], bf16, kind="ExternalInput")
    bout = nc.dram_tensor("bout2", [tpc, nlbl], f32, kind="ExternalInput")
    out = nc.dram_tensor("out", [tpc, nlbl], f32, kind="ExternalOutput")

    assert depth >= 4

    SIG, TANH = AF.Sigmoid, AF.Tanh
    BIDX = {"i": 0, "f": 1, "o": 2, "u": 3}

    with tile.TileContext(nc) as tc:
        import contextlib
        with contextlib.ExitStack() as ctx:
            wp = ctx.enter_context(tc.tile_pool(name="wp", bufs=1))
            stream = ctx.enter_context(tc.tile_pool(name="stream", bufs=2))
            tmp = ctx.enter_context(tc.tile_pool(name="tmp", bufs=2))
            acc = ctx.enter_context(tc.tile_pool(name="acc", bufs=2))
            chk = ctx.enter_context(tc.tile_pool(name="chk", bufs=2))
            hcp = ctx.enter_context(tc.tile_pool(name="hcp", bufs=1))
            psum = ctx.enter_context(
                tc.tile_pool(name="psum", bufs=2, space="PSUM"))

            GMAX = 2048  # max columns per gate/f group (4 PSUM banks)

            # ---- load constants ----
            wall_sb = wp.tile([P, len(wnames), P], bf16, name="wall_sb",
                              tag="wall_sb")
            nc.sync.dma_start(out=wall_sb, in_=wall[:])
            wsb = {nm: wall_sb[:, j, :] for j, nm in enumerate(wnames)}
            bias_sb = wp.tile([P, 4], f32, name="bias_sb", tag="bias_sb")
            nc.sync.dma_start(out=bias_sb, in_=bias4[:])
            woutT_sb = wp.tile([P, nlbl], bf16, name="woutT_sb", tag="woutT_sb")
            nc.sync.dma_start(out=woutT_sb, in_=woutT[:])
            bout_sb = wp.tile([tpc, nlbl], f32, name="bout_sb", tag="bout_sb")
            nc.sync.dma_start(out=bout_sb, in_=bout[:])

            def bias_ap(g):
                i = BIDX[g]
                return bias_sb[:, i:i + 1]

            # persistent h/c for resident levels (leaves and level
            # depth-2 are streamed/consumed in flight, never stored whole)
            hres, cres = {}, {}
            for l in range(depth - 2):
                hres[l] = hcp.tile([P, n[l]], bf16, name=f"h{l}_sb",
                                   tag=f"h{l}_sb")
                cres[l] = hcp.tile([P, n[l]], f32, name=f"c{l}_sb",
                                   tag=f"c{l}_sb")

            def matmul_group(ps, w0, rhs0, w1=None, rhs1=None, G=GMAX):
                """ps[:, :G] = w0.T@rhs0 (+ w1.T@rhs1).  512-col banks."""
                nb = (G + 511) // 512
                for b in range(nb):
                    s = b * 512
                    e = min(s + 512, G)
                    nc.tensor.matmul(ps[:, s:e], wsb[w0], rhs0[:, s:e],
                                     start=True, stop=(w1 is None))
                if w1 is not None:
                    for b in range(nb):
                        s = b * 512
                        e = min(s + 512, G)
                        nc.tensor.matmul(ps[:, s:e], wsb[w1], rhs1[:, s:e],
                                         start=False, stop=True)

            def gate_pass(xt_ap, hs_ap, fcs_ap, h_out, c_out, G):
                """Compute i,u,o gates + c,h for G parent columns."""
                leaf = hs_ap is None
                g_sb = {}
                for gname, wx, wh, func in (("i", "ix", "ih", SIG),
                                            ("u", "ux", "uh", TANH),
                                            ("o", "ox", "oh", SIG)):
                    ps = psum.tile([P, GMAX], f32, name=f"ps_{gname}",
                                   tag="ps")
                    if leaf:
                        matmul_group(ps, wx, xt_ap, G=G)
                    else:
                        matmul_group(ps, wx, xt_ap, wh, hs_ap, G=G)
                    gdt = bf16 if gname == "o" else f32
                    g = tmp.tile([P, G], gdt, name=f"g_{gname}",
                                 tag=f"g_{gname}")
                    nc.scalar.activation(out=g, in_=ps[:, :G], func=func,
                                         bias=bias_ap(gname), scale=1.0)
                    g_sb[gname] = g
                nc.vector.tensor_mul(c_out, g_sb["i"], g_sb["u"])
                if fcs_ap is not None:
                    nc.vector.tensor_add(c_out, c_out, fcs_ap)
                tt = tmp.tile([P, G], bf16, name="tt", tag="tt")
                nc.scalar.activation(out=tt, in_=c_out, func=TANH,
                                     bias=0.0, scale=1.0)
                nc.vector.tensor_mul(h_out, g_sb["o"], tt)

            def f_pass(xb_ap, hch_ap, cch_ap, fcs_out, hs_out, S):
                """f gates for S child cols; reduce fc and h by groups of 4."""
                ps = psum.tile([P, GMAX], f32, name="ps_f", tag="ps")
                matmul_group(ps, "fx", xb_ap, "fh", hch_ap, G=S)
                f = tmp.tile([P, S], f32, name="g_f", tag="g_f")
                nc.scalar.activation(out=f, in_=ps[:, :S], func=SIG,
                                     bias=bias_ap("f"), scale=1.0)
                fc = tmp.tile([P, S], f32, name="fc", tag="fc")
                nc.vector.tensor_mul(fc, f, cch_ap)
                # pairwise reduce x4 -> fcs_out  (VectorE)
                fcv = fc.rearrange("p (a two) -> p a two", two=2)
                rt = tmp.tile([P, S // 2], f32, name="rt", tag="rt",
                              bufs=1)
                nc.vector.tensor_add(rt, fcv[:, :, 0], fcv[:, :, 1])
                rtv = rt.rearrange("p (a two) -> p a two", two=2)
                nc.vector.tensor_add(fcs_out, rtv[:, :, 0], rtv[:, :, 1])
                # pairwise reduce h x4 -> hs_out  (GpSimd)
                hv = hch_ap.rearrange("p (a two) -> p a two", two=2)
                rt2 = tmp.tile([P, S // 2], bf16, name="rt2", tag="rt2",
                               bufs=1)
                nc.gpsimd.tensor_tensor(rt2, hv[:, :, 0], hv[:, :, 1], ADD)
                rt2v = rt2.rearrange("p (a two) -> p a two", two=2)
                nc.gpsimd.tensor_tensor(hs_out, rt2v[:, :, 0], rt2v[:, :, 1],
                                        ADD)

            def load_stream(tag, src, a, b, dt=bf16):
                t = stream.tile([P, b - a], dt, name=tag, tag=tag)
                nc.sync.dma_start(out=t, in_=src[:, a:b])
                return t

            # ================= leaves fused with level depth-2 ============
            l6 = depth - 2
            GQ = min(GMAX, n[l6])          # parents per level-l6 gate group
            cpq = max(1, (GQ * BR) // ch_leaf)   # leaf chunks per gate group
            assert GQ * BR == cpq * ch_leaf
            nchunks = nleaf // ch_leaf
            fcs = hs = None
            pending = None
            l5 = depth - 3
            l4 = depth - 4
            q5 = GQ // BR
            q4 = max(1, q5 // BR)
            # level depth-3 / depth-4 child-sum accumulators
            fcs5 = acc.tile([P, n[l5]], f32, name="fcs5", tag="fcs5", bufs=1)
            hs5 = acc.tile([P, n[l5]], bf16, name="hs5", tag="hs5", bufs=1)
            fcs4 = acc.tile([P, n[l4]], f32, name="fcs4", tag="fcs4", bufs=1)
            hs4 = acc.tile([P, n[l4]], bf16, name="hs4", tag="hs4", bufs=1)

            def emit_l6_gates(xt6, hs_p, fcs_p, p0):
                h6o = chk.tile([P, GQ], bf16, name="h6o", tag="h6o")
                c6o = chk.tile([P, GQ], f32, name="c6o", tag="c6o")
                gate_pass(xt6, hs_p, fcs_p, h6o, c6o, GQ)
                return (h6o, c6o, p0)

            def emit_l5_f(h6o, c6o, p0):
                # f-pass of level depth-3 over the group produced above
                xbt5 = load_stream("xb", xb, off[l6] + p0, off[l6] + p0 + GQ)
                a5 = p0 // BR
                f_pass(xbt5, h6o, c6o, fcs5[:, a5:a5 + q5],
                       hs5[:, a5:a5 + q5], GQ)

            def emit_l5_gates(a5):
                xt5g = load_stream("xt", xT, off[l5] + a5, off[l5] + a5 + q5)
                gate_pass(xt5g, hs5[:, a5:a5 + q5], fcs5[:, a5:a5 + q5],
                          hres[l5][:, a5:a5 + q5], cres[l5][:, a5:a5 + q5],
                          q5)

            def emit_l4_f(a5):
                # children: level depth-3 cols [a5, a5+q5)
                xbt4 = load_stream("xb", xb, off[l5] + a5, off[l5] + a5 + q5)
                a4 = a5 // BR
                f_pass(xbt4, hres[l5][:, a5:a5 + q5], cres[l5][:, a5:a5 + q5],
                       fcs4[:, a4:a4 + q4], hs4[:, a4:a4 + q4], q5)

            pending_l5f = None
            pending_l5g = None
            pending_l4f = None
            for k in range(nchunks):
                xt7 = load_stream("xt", xT, off[lleaf] + k * ch_leaf,
                                  off[lleaf] + (k + 1) * ch_leaf)
                h7k = stream.tile([P, ch_leaf], bf16, name="hch", tag="hch")
                c7k = stream.tile([P, ch_leaf], f32, name="cch", tag="cch")
                gate_pass(xt7, None, None, h7k, c7k, ch_leaf)
                # deferred work from previous groups (one stage per chunk):
                # ACT chews the leaf sigmoids while PE runs these matmuls
                if pending_l4f is not None:
                    emit_l4_f(pending_l4f)
                    pending_l4f = None
                if pending_l5g is not None:
                    emit_l5_gates(pending_l5g)
                    pending_l4f = pending_l5g
                    pending_l5g = None
                if pending_l5f is not None:
                    emit_l5_f(*pending_l5f)
                    pending_l5g = pending_l5f[2] // BR
                    pending_l5f = None
                if pending is not None:
                    pending_l5f = emit_l6_gates(*pending)
                    pending = None
                # f-pass of level l6 over this chunk's children
                xbt = load_stream("xb", xb, off[lleaf] + k * ch_leaf,
                                  off[lleaf] + (k + 1) * ch_leaf)
                jq = k % cpq
                if jq == 0:
                    fcs = acc.tile([P, GQ], f32, name="fcs", tag="fcs")
                    hs = acc.tile([P, GQ], bf16, name="hs", tag="hs")
                q = ch_leaf // BR
                f_pass(xbt, h7k, c7k, fcs[:, jq * q:(jq + 1) * q],
                       hs[:, jq * q:(jq + 1) * q], ch_leaf)
                if jq == cpq - 1:
                    gq_i = k // cpq
                    p0 = gq_i * GQ
                    xt6 = load_stream("xt", xT, off[l6] + p0,
                                      off[l6] + p0 + GQ)
                    pending = (xt6, hs, fcs, p0)
            # drain the software pipeline
            if pending is not None:
                pending_l5f = emit_l6_gates(*pending)
                pending = None
            if pending_l5f is not None:
                emit_l5_f(*pending_l5f)
                pending_l5g = pending_l5f[2] // BR
                pending_l5f = None
            if pending_l5g is not None:
                emit_l5_gates(pending_l5g)
                pending_l4f = pending_l5g
                pending_l5g = None
            if pending_l4f is not None:
                emit_l4_f(pending_l4f)
                pending_l4f = None

            # gates of level depth-4 (its child-sums are complete now)
            xt4 = load_stream("xt", xT, off[l4], off[l4] + n[l4])
            gate_pass(xt4, hs4, fcs4, hres[l4], cres[l4], n[l4])

            # ================= levels depth-5 .. 0 ========================
            for l in range(depth - 5, -1, -1):
                npar, nch = n[l], n[l + 1]
                S = min(GMAX, nch)
                ngr = nch // S
                fcs = acc.tile([P, npar], f32, name="fcs", tag="fcs")
                hs = acc.tile([P, npar], bf16, name="hs", tag="hs")
                for g in range(ngr):
                    a = g * S
                    hch = hres[l + 1][:, a:a + S]
                    cch = cres[l + 1][:, a:a + S]
                    xbt = load_stream("xb", xb, off[l + 1] + a,
                                      off[l + 1] + a + S)
                    q = S // BR
                    f_pass(xbt, hch, cch, fcs[:, g * q:(g + 1) * q],
                           hs[:, g * q:(g + 1) * q], S)
                xt = load_stream("xt", xT, off[l], off[l] + npar)
                gate_pass(xt, hs, fcs, hres[l], cres[l], npar)

            # ================= output head ================================
            ps = psum.tile([P, GMAX], f32, name="ps_out", tag="ps")
            nc.tensor.matmul(ps[:tpc, :nlbl], hres[0], woutT_sb,
                             start=True, stop=True)
            out_sb = tmp.tile([tpc, nlbl], f32, name="out_sb", tag="out_sb")
            nc.vector.tensor_add(out_sb, ps[:tpc, :nlbl], bout_sb)
            nc.sync.dma_start(out=out[:], in_=out_sb)

    if legalize:
        _legalize_waits(nc)
    return nc


def _prep_core_inputs(embeds, weights, tpc, depth, n_cores):
    """Host-side shard + transpose + parent-broadcast per core."""
    n, off, ntot = _levels(tpc, depth)
    T = tpc * n_cores
    counts = [T * BR**l for l in range(depth)]
    offsets = [0]
    for c in counts:
        offsets.append(offsets[-1] + c)

    common = dict(weights)
    in_maps = []
    for d in range(n_cores):
        import ml_dtypes
        bf16 = ml_dtypes.bfloat16
        shard = np.concatenate(
            [embeds[offsets[l] + tpc * d * BR**l:
                    offsets[l] + tpc * (d + 1) * BR**l] for l in range(depth)],
            axis=0)
        xT = np.ascontiguousarray(shard.T.astype(bf16))   # [P, ntot]
        xbm = np.zeros_like(xT)
        for l in range(1, depth):
            xbm[:, off[l]:off[l + 1]] = np.repeat(
                xT[:, off[l - 1]:off[l]], BR, axis=1)
        m = {"xT": xT, "xb": xbm}
        m.update(common)
        in_maps.append(m)
    return in_maps


def _prep_weights(Wix, bix, Wih, Wfx, bfx, Wfh, Wox, box, Woh, Wux, bux, Wuh,
                  Wout, bout, tpc):
    import ml_dtypes
    f = np.float32
    bf = ml_dtypes.bfloat16
    # order must match build_nc's wnames: ix, ih, fx, fh, ox, oh, ux, uh
    wall = np.stack([Wix.T, Wih.T, Wfx.T, Wfh.T, Wox.T, Woh.T, Wux.T, Wuh.T],
                    axis=1)   # [128 (in-feat), 8, 128 (out-feat)]
    w = {
        "Wall": np.ascontiguousarray(wall, dtype=bf),
        "bias4": np.ascontiguousarray(
            np.stack([bix, bfx, box, bux], axis=1), dtype=f),
        "WoutT": np.ascontiguousarray(Wout.T, dtype=bf),
        "bout2": np.ascontiguousarray(np.tile(bout, (tpc, 1)), dtype=f),
    }
    return w


def _ensure_ntff_hook():
    """The RL container's antenv lacks axon_hooks; install a shim and
    register the ctypes NTFF profiler so trace=True works."""
    import types

    try:
        from antenv.axon_hooks import get_axon_ntff_profile_hook  # noqa
        return
    except ImportError:
        pass
    mod = types.ModuleType("antenv.axon_hooks")
    _h = [None]
    mod.set_axon_ntff_profile_hook = lambda h: _h.__setitem__(0, h)
    mod.get_axon_ntff_profile_hook = lambda: _h[0]
    sys.modules["antenv.axon_hooks"] = mod
    import antenv
    antenv.axon_hooks = mod
    try:
        from trn_agent_boot.trn_boot import _ntff_profile_via_ctypes
        h = _ntff_profile_via_ctypes("/opt/axon/libaxon_pjrt.so")
        if h is not None:
            mod.set_axon_ntff_profile_hook(h)
    except Exception:
        pass


def kernel(embeds, Wix, bix, Wih, Wfx, bfx, Wfh, Wox, box, Woh, Wux, bux, Wuh,
           Wout, bout, _trace=False):
    from concourse import bass_utils
    from concourse.bass_utils import run_bass_kernel_spmd

    if _trace:
        _ensure_ntff_hook()
        bass_utils.upload_artifacts = lambda d: d  # no S3 in this container

    n_cores = 8
    depth = 8
    T = 16
    tpc = T // n_cores

    embeds = np.asarray(embeds, dtype=np.float32)
    weights = _prep_weights(
        np.asarray(Wix), np.asarray(bix), np.asarray(Wih), np.asarray(Wfx),
        np.asarray(bfx), np.asarray(Wfh), np.asarray(Wox), np.asarray(box),
        np.asarray(Woh), np.asarray(Wux), np.asarray(bux), np.asarray(Wuh),
        np.asarray(Wout), np.asarray(bout), tpc)
    in_maps = _prep_core_inputs(embeds, weights, tpc, depth, n_cores)

    key = (tpc, depth)
    if key not in _NC_CACHE:
        _NC_CACHE[key] = build_nc(tpc=tpc, depth=depth)
    nc = _NC_CACHE[key]

    res = run_bass_kernel_spmd(nc, in_maps, core_ids=list(range(n_cores)),
                               trace=_trace)
    outs = np.concatenate([r["out"] for r in res.results], axis=0)
    if _trace:
        kernel.last_results = res
    return outs.astype(np.float32)


kernel.last_results = None


# revision 20
# speedup vs baseline: 1.0719x; 1.0020x over previous
"""BatchChildSumTreeLSTM Trainium2 kernel.

Forest of T complete B-ary trees, level-major node layout, processed
leaves-first.  Sharded across 8 NeuronCores by tree (2 trees / core);
every level's per-core slice is contiguous, so the whole recursion is
device-local.

Per-core layout is feature-major: activations live in SBUF as
[128 features (partitions), n nodes (free)].  Matmuls are
out[H, n] = W.T.T @ rhs with pre-transposed weights as lhsT, so matmul
outputs are directly consumable as later matmul inputs and PSUM
accumulates the x-projection with the h-recurrence.  The host feeds:
  xT  [128, ntot]  embeds transposed (per-core shard)
  xb  [128, ntot]  parent embedding broadcast to each child column
                   (level-0 columns unused) -> f-gate is 2 matmuls.
Gates are evaluated by ScalarE straight out of PSUM with per-partition
bias APs; VectorE does the elementwise muls and the fc child-sum
(pairwise tree); GpSimd does the h child-sum.  Level 6 h/c spill to
DRAM (too big for SBUF together with the level-7 streaming chunks);
levels <= 5 stay resident.
"""

import sys

if "/opt/trn_rl_repo" not in sys.path:
    sys.path.insert(0, "/opt/trn_rl_repo")

import numpy as np

P = 128          # feature dim == partitions
BR = 4           # branching factor
NLBL = 5

F32 = None       # filled lazily (mybir import is heavy)

_NC_CACHE = {}


def _levels(tpc, depth):
    n = [tpc * BR**l for l in range(depth)]
    off = [0]
    for c in n:
        off.append(off[-1] + c)
    return n, off, off[-1]


def _legalize_waits(nc, max_waits=1):
    """This walrus build accepts at most one sync-wait command per
    instruction (any type).  Hoist excess waits onto same-engine NoOps
    inserted right before the instruction; engine program order makes
    this exactly equivalent."""
    import concourse.mybir as mybir

    n_nops = 0
    for fn in nc.m.functions:
        for blk in fn.blocks:
            new_insts = []
            for inst in blk.instructions:
                si = getattr(inst, "sync_info", None)
                if si is not None and si.on_wait and len(si.on_wait) > max_waits:
                    waits = list(si.on_wait)
                    hoist, keep = waits[:-max_waits], waits[-max_waits:]
                    eng = getattr(inst, "engine", None)
                    for j, w in enumerate(hoist):
                        nop = mybir.InstNoOp(
                            name=f"{inst.name}-wn{j}",
                            engine=eng,
                            bass_nofuse=True,
                        )
                        nop.sync_info = mybir.SyncInfo(on_wait=[w],
                                                       on_update=[])
                        new_insts.append(nop)
                        n_nops += 1
                    inst.sync_info = mybir.SyncInfo(
                        on_wait=keep, on_update=list(si.on_update))
                new_insts.append(inst)
            blk.instructions = new_insts
    return n_nops


def build_nc(tpc=2, depth=8, ch_leaf=2048, resident_max=2048, nlbl=NLBL,
             legalize=True):
    """Build the per-core Bass/Tile program (identical on all cores)."""
    import concourse.bass as bass
    import concourse.mybir as mybir
    import concourse.tile as tile

    f32 = mybir.dt.float32
    bf16 = mybir.dt.bfloat16
    AF = mybir.ActivationFunctionType
    ADD = mybir.AluOpType.add

    n, off, ntot = _levels(tpc, depth)
    lleaf = depth - 1
    nleaf = n[lleaf]
    ch_leaf = min(ch_leaf, nleaf)
    assert nleaf % ch_leaf == 0

    nc = bass.Bass()

    xT = nc.dram_tensor("xT", [P, ntot], bf16, kind="ExternalInput")
    xb = nc.dram_tensor("xb", [P, ntot], bf16, kind="ExternalInput")
    wnames = ["ix", "ih", "fx", "fh", "ox", "oh", "ux", "uh"]
    wall = nc.dram_tensor("Wall", [P, len(wnames), P], bf16,
                          kind="ExternalInput")
    bias4 = nc.dram_tensor("bias4", [P, 4], f32, kind="ExternalInput")
    woutT = nc.dram_tensor("WoutT", [P, nlbl], bf16, kind="ExternalInput")
    bout = nc.dram_tensor("bout2", [tpc, nlbl], f32, kind="ExternalInput")
    out = nc.dram_tensor("out", [tpc, nlbl], f32, kind="ExternalOutput")

    assert depth >= 4

    SIG, TANH = AF.Sigmoid, AF.Tanh
    BIDX = {"i": 0, "f": 1, "o": 2, "u": 3}

    with tile.TileContext(nc) as tc:
        import contextlib
        with contextlib.ExitStack() as ctx:
            wp = ctx.enter_context(tc.tile_pool(name="wp", bufs=1))
            stream = ctx.enter_context(tc.tile_pool(name="stream", bufs=2))
            tmp = ctx.enter_context(tc.tile_pool(name="tmp", bufs=2))
            acc = ctx.enter_context(tc.tile_pool(name="acc", bufs=2))
            chk = ctx.enter_context(tc.tile_pool(name="chk", bufs=2))
            hcp = ctx.enter_context(tc.tile_pool(name="hcp", bufs=1))
            psum = ctx.enter_context(
                tc.tile_pool(name="psum", bufs=2, space="PSUM"))

            GMAX = 2048  # max columns per gate/f group (4 PSUM banks)

            # ---- load constants ----
            wall_sb = wp.tile([P, len(wnames), P], bf16, name="wall_sb",
                              tag="wall_sb")
            nc.sync.dma_start(out=wall_sb, in_=wall[:])
            wsb = {nm: wall_sb[:, j, :] for j, nm in enumerate(wnames)}
            bias_sb = wp.tile([P, 4], f32, name="bias_sb", tag="bias_sb")
            nc.sync.dma_start(out=bias_sb, in_=bias4[:])
            woutT_sb = wp.tile([P, nlbl], bf16, name="woutT_sb", tag="woutT_sb")
            nc.sync.dma_start(out=woutT_sb, in_=woutT[:])
            bout_sb = wp.tile([tpc, nlbl], f32, name="bout_sb", tag="bout_sb")
            nc.sync.dma_start(out=bout_sb, in_=bout[:])

            def bias_ap(g):
                i = BIDX[g]
                return bias_sb[:, i:i + 1]

            # persistent h/c for resident levels (leaves and level
            # depth-2 are streamed/consumed in flight, never stored whole)
            hres, cres = {}, {}
            for l in range(depth - 2):
                hres[l] = hcp.tile([P, n[l]], bf16, name=f"h{l}_sb",
                                   tag=f"h{l}_sb")
                cres[l] = hcp.tile([P, n[l]], f32, name=f"c{l}_sb",
                                   tag=f"c{l}_sb")

            def matmul_group(ps, w0, rhs0, w1=None, rhs1=None, G=GMAX):
                """ps[:, :G] = w0.T@rhs0 (+ w1.T@rhs1).  512-col banks."""
                nb = (G + 511) // 512
                for b in range(nb):
                    s = b * 512
                    e = min(s + 512, G)
                    nc.tensor.matmul(ps[:, s:e], wsb[w0], rhs0[:, s:e],
                                     start=True, stop=(w1 is None))
                if w1 is not None:
                    for b in range(nb):
                        s = b * 512
                        e = min(s + 512, G)
                        nc.tensor.matmul(ps[:, s:e], wsb[w1], rhs1[:, s:e],
                                         start=False, stop=True)

            def gate_pass(xt_ap, hs_ap, fcs_ap, h_out, c_out, G):
                """Compute i,u,o gates + c,h for G parent columns."""
                leaf = hs_ap is None
                g_sb = {}
                for gname, wx, wh, func in (("i", "ix", "ih", SIG),
                                            ("u", "ux", "uh", TANH),
                                            ("o", "ox", "oh", SIG)):
                    ps = psum.tile([P, GMAX], f32, name=f"ps_{gname}",
                                   tag="ps")
                    if leaf:
                        matmul_group(ps, wx, xt_ap, G=G)
                    else:
                        matmul_group(ps, wx, xt_ap, wh, hs_ap, G=G)
                    gdt = bf16 if gname == "o" else f32
                    g = tmp.tile([P, G], gdt, name=f"g_{gname}",
                                 tag=f"g_{gname}")
                    nc.scalar.activation(out=g, in_=ps[:, :G], func=func,
                                         bias=bias_ap(gname), scale=1.0)
                    g_sb[gname] = g
                nc.vector.tensor_mul(c_out, g_sb["i"], g_sb["u"])
                if fcs_ap is not None:
                    nc.vector.tensor_add(c_out, c_out, fcs_ap)
                tt = tmp.tile([P, G], bf16, name="tt", tag="tt")
                nc.scalar.activation(out=tt, in_=c_out, func=TANH,
                                     bias=0.0, scale=1.0)
                nc.vector.tensor_mul(h_out, g_sb["o"], tt)

            def f_pass(xb_ap, hch_ap, cch_ap, fcs_out, hs_out, S):
                """f gates for S child cols; reduce fc and h by groups of 4."""
                ps = psum.tile([P, GMAX], f32, name="ps_f", tag="ps")
                matmul_group(ps, "fx", xb_ap, "fh", hch_ap, G=S)
                f = tmp.tile([P, S], f32, name="g_f", tag="g_f", bufs=1)
                nc.scalar.activation(out=f, in_=ps[:, :S], func=SIG,
                                     bias=bias_ap("f"), scale=1.0)
                fc = tmp.tile([P, S], f32, name="fc", tag="fc", bufs=1)
                nc.vector.tensor_mul(fc, f, cch_ap)
                # pairwise reduce x4 -> fcs_out  (VectorE)
                fcv = fc.rearrange("p (a two) -> p a two", two=2)
                rt = tmp.tile([P, S // 2], f32, name="rt", tag="rt",
                              bufs=1)
                nc.vector.tensor_add(rt, fcv[:, :, 0], fcv[:, :, 1])
                rtv = rt.rearrange("p (a two) -> p a two", two=2)
                nc.vector.tensor_add(fcs_out, rtv[:, :, 0], rtv[:, :, 1])
                # pairwise reduce h x4 -> hs_out  (GpSimd)
                hv = hch_ap.rearrange("p (a two) -> p a two", two=2)
                rt2 = tmp.tile([P, S // 2], bf16, name="rt2", tag="rt2",
                               bufs=1)
                nc.gpsimd.tensor_tensor(rt2, hv[:, :, 0], hv[:, :, 1], ADD)
                rt2v = rt2.rearrange("p (a two) -> p a two", two=2)
                nc.gpsimd.tensor_tensor(hs_out, rt2v[:, :, 0], rt2v[:, :, 1],
                                        ADD)

            _stream_bufs = {"xt": 3, "xb": 4}

            def load_stream(tag, src, a, b, dt=bf16, bufs=None):
                t = stream.tile([P, b - a], dt, name=tag, tag=tag,
                                bufs=_stream_bufs.get(tag, 2))
                nc.sync.dma_start(out=t, in_=src[:, a:b])
                return t

            # ================= leaves fused with level depth-2 ============
            l6 = depth - 2
            GQ = min(GMAX, n[l6])          # parents per level-l6 gate group
            cpq = max(1, (GQ * BR) // ch_leaf)   # leaf chunks per gate group
            assert GQ * BR == cpq * ch_leaf
            nchunks = nleaf // ch_leaf
            fcs = hs = None
            pending = None
            l5 = depth - 3
            l4 = depth - 4
            q5 = GQ // BR
            q4 = max(1, q5 // BR)
            # level depth-3 / depth-4 child-sum accumulators
            fcs5 = acc.tile([P, n[l5]], f32, name="fcs5", tag="fcs5", bufs=1)
            hs5 = acc.tile([P, n[l5]], bf16, name="hs5", tag="hs5", bufs=1)
            fcs4 = acc.tile([P, n[l4]], f32, name="fcs4", tag="fcs4", bufs=1)
            hs4 = acc.tile([P, n[l4]], bf16, name="hs4", tag="hs4", bufs=1)

            def emit_l6_gates(xt6, hs_p, fcs_p, p0):
                h6o = chk.tile([P, GQ], bf16, name="h6o", tag="h6o")
                c6o = chk.tile([P, GQ], f32, name="c6o", tag="c6o")
                gate_pass(xt6, hs_p, fcs_p, h6o, c6o, GQ)
                return (h6o, c6o, p0)

            def emit_l5_f(h6o, c6o, p0):
                # f-pass of level depth-3 over the group produced above
                xbt5 = load_stream("xb", xb, off[l6] + p0, off[l6] + p0 + GQ)
                a5 = p0 // BR
                f_pass(xbt5, h6o, c6o, fcs5[:, a5:a5 + q5],
                       hs5[:, a5:a5 + q5], GQ)

            def emit_l5_gates(a5):
                xt5g = load_stream("xt", xT, off[l5] + a5, off[l5] + a5 + q5)
                gate_pass(xt5g, hs5[:, a5:a5 + q5], fcs5[:, a5:a5 + q5],
                          hres[l5][:, a5:a5 + q5], cres[l5][:, a5:a5 + q5],
                          q5)

            def emit_l4_f(a5):
                # children: level depth-3 cols [a5, a5+q5)
                xbt4 = load_stream("xb", xb, off[l5] + a5, off[l5] + a5 + q5)
                a4 = a5 // BR
                f_pass(xbt4, hres[l5][:, a5:a5 + q5], cres[l5][:, a5:a5 + q5],
                       fcs4[:, a4:a4 + q4], hs4[:, a4:a4 + q4], q5)

            pending_l5f = None
            pending_l5g = None
            pending_l4f = None
            pending_f = None
            q = ch_leaf // BR

            def emit_leaf_f(xbt_p, h7_p, c7_p, kp):
                # f-pass of level l6 over chunk kp's children (one chunk old,
                # so h/c are long since ready and PE never stalls on them)
                nonlocal fcs, hs, pending
                jq = kp % cpq
                if jq == 0:
                    fcs = acc.tile([P, GQ], f32, name="fcs", tag="fcs")
                    hs = acc.tile([P, GQ], bf16, name="hs", tag="hs")
                f_pass(xbt_p, h7_p, c7_p, fcs[:, jq * q:(jq + 1) * q],
                       hs[:, jq * q:(jq + 1) * q], ch_leaf)
                if jq == cpq - 1:
                    gq_i = kp // cpq
                    p0 = gq_i * GQ
                    xt6 = load_stream("xt", xT, off[l6] + p0,
                                      off[l6] + p0 + GQ)
                    pending = (xt6, hs, fcs, p0)

            for k in range(nchunks):
                xt7 = load_stream("xt", xT, off[lleaf] + k * ch_leaf,
                                  off[lleaf] + (k + 1) * ch_leaf)
                h7k = stream.tile([P, ch_leaf], bf16, name="hch", tag="hch")
                c7k = stream.tile([P, ch_leaf], f32, name="cch", tag="cch")
                gate_pass(xt7, None, None, h7k, c7k, ch_leaf)
                # deferred work from previous chunks/groups: ACT chews the
                # leaf sigmoids while PE runs these matmuls on old data
                if pending_l4f is not None:
                    emit_l4_f(pending_l4f)
                    pending_l4f = None
                if pending_l5g is not None:
                    emit_l5_gates(pending_l5g)
                    pending_l4f = pending_l5g
                    pending_l5g = None
                if pending_l5f is not None:
                    emit_l5_f(*pending_l5f)
                    pending_l5g = pending_l5f[2] // BR
                    pending_l5f = None
                if pending is not None:
                    pending_l5f = emit_l6_gates(*pending)
                    pending = None
                xbt = load_stream("xb", xb, off[lleaf] + k * ch_leaf,
                                  off[lleaf] + (k + 1) * ch_leaf)
                emit_leaf_f(xbt, h7k, c7k, k)
            # drain the software pipeline
            if pending is not None:
                pending_l5f = emit_l6_gates(*pending)
                pending = None
            if pending_l5f is not None:
                emit_l5_f(*pending_l5f)
                pending_l5g = pending_l5f[2] // BR
                pending_l5f = None
            if pending_l5g is not None:
                emit_l5_gates(pending_l5g)
                pending_l4f = pending_l5g
                pending_l5g = None
            if pending_l4f is not None:
                emit_l4_f(pending_l4f)
                pending_l4f = None

            # gates of level depth-4 (its child-sums are complete now)
            xt4 = load_stream("xt", xT, off[l4], off[l4] + n[l4])
            gate_pass(xt4, hs4, fcs4, hres[l4], cres[l4], n[l4])

            # ================= levels depth-5 .. 0 ========================
            for l in range(depth - 5, -1, -1):
                npar, nch = n[l], n[l + 1]
                S = min(GMAX, nch)
                ngr = nch // S
                fcs = acc.tile([P, npar], f32, name="fcs", tag="fcs")
                hs = acc.tile([P, npar], bf16, name="hs", tag="hs")
                for g in range(ngr):
                    a = g * S
                    hch = hres[l + 1][:, a:a + S]
                    cch = cres[l + 1][:, a:a + S]
                    xbt = load_stream("xb", xb, off[l + 1] + a,
                                      off[l + 1] + a + S)
                    q = S // BR
                    f_pass(xbt, hch, cch, fcs[:, g * q:(g + 1) * q],
                           hs[:, g * q:(g + 1) * q], S)
                xt = load_stream("xt", xT, off[l], off[l] + npar)
                gate_pass(xt, hs, fcs, hres[l], cres[l], npar)

            # ================= output head ================================
            ps = psum.tile([P, GMAX], f32, name="ps_out", tag="ps")
            nc.tensor.matmul(ps[:tpc, :nlbl], hres[0], woutT_sb,
                             start=True, stop=True)
            out_sb = tmp.tile([tpc, nlbl], f32, name="out_sb", tag="out_sb")
            nc.vector.tensor_add(out_sb, ps[:tpc, :nlbl], bout_sb)
            nc.sync.dma_start(out=out[:], in_=out_sb)

    if legalize:
        _legalize_waits(nc)
    return nc


def _prep_core_inputs(embeds, weights, tpc, depth, n_cores):
    """Host-side shard + transpose + parent-broadcast per core."""
    n, off, ntot = _levels(tpc, depth)
    T = tpc * n_cores
    counts = [T * BR**l for l in range(depth)]
    offsets = [0]
    for c in counts:
        offsets.append(offsets[-1] + c)

    common = dict(weights)
    in_maps = []
    for d in range(n_cores):
        import ml_dtypes
        bf16 = ml_dtypes.bfloat16
        shard = np.concatenate(
            [embeds[offsets[l] + tpc * d * BR**l:
                    offsets[l] + tpc * (d + 1) * BR**l] for l in range(depth)],
            axis=0)
        xT = np.ascontiguousarray(shard.T.astype(bf16))   # [P, ntot]
        xbm = np.zeros_like(xT)
        for l in range(1, depth):
            xbm[:, off[l]:off[l + 1]] = np.repeat(
                xT[:, off[l - 1]:off[l]], BR, axis=1)
        m = {"xT": xT, "xb": xbm}
        m.update(common)
        in_maps.append(m)
    return in_maps


def _prep_weights(Wix, bix, Wih, Wfx, bfx, Wfh, Wox, box, Woh, Wux, bux, Wuh,
                  Wout, bout, tpc):
    import ml_dtypes
    f = np.float32
    bf = ml_dtypes.bfloat16
    # order must match build_nc's wnames: ix, ih, fx, fh, ox, oh, ux, uh
    wall = np.stack([Wix.T, Wih.T, Wfx.T, Wfh.T, Wox.T, Woh.T, Wux.T, Wuh.T],
                    axis=1)   # [128 (in-feat), 8, 128 (out-feat)]
    w = {
        "Wall": np.ascontiguousarray(wall, dtype=bf),
        "bias4": np.ascontiguousarray(
            np.stack([bix, bfx, box, bux], axis=1), dtype=f),
        "WoutT": np.ascontiguousarray(Wout.T, dtype=bf),
        "bout2": np.ascontiguousarray(np.tile(bout, (tpc, 1)), dtype=f),
    }
    return w


def _ensure_ntff_hook():
    """The RL container's antenv lacks axon_hooks; install a shim and
    register the ctypes NTFF profiler so trace=True works."""
    import types

    try:
        from antenv.axon_hooks import get_axon_ntff_profile_hook  # noqa
        return
    except ImportError:
        pass
    mod = types.ModuleType("antenv.axon_hooks")
    _h = [None]
    mod.set_axon_ntff_profile_hook = lambda h: _h.__setitem__(0, h)
    mod.get_axon_ntff_profile_hook = lambda: _h[0]
    sys.modules["antenv.axon_hooks"] = mod
    import antenv
    antenv.axon_hooks = mod
    try:
        from trn_agent_boot.trn_boot import _ntff_profile_via_ctypes
        h = _ntff_profile_via_ctypes("/opt/axon/libaxon_pjrt.so")
        if h is not None:
            mod.set_axon_ntff_profile_hook(h)
    except Exception:
        pass


def kernel(embeds, Wix, bix, Wih, Wfx, bfx, Wfh, Wox, box, Woh, Wux, bux, Wuh,
           Wout, bout, _trace=False):
    from concourse import bass_utils
    from concourse.bass_utils import run_bass_kernel_spmd

    if _trace:
        _ensure_ntff_hook()
        bass_utils.upload_artifacts = lambda d: d  # no S3 in this container

    n_cores = 8
    depth = 8
    T = 16
    tpc = T // n_cores

    embeds = np.asarray(embeds, dtype=np.float32)
    weights = _prep_weights(
        np.asarray(Wix), np.asarray(bix), np.asarray(Wih), np.asarray(Wfx),
        np.asarray(bfx), np.asarray(Wfh), np.asarray(Wox), np.asarray(box),
        np.asarray(Woh), np.asarray(Wux), np.asarray(bux), np.asarray(Wuh),
        np.asarray(Wout), np.asarray(bout), tpc)
    in_maps = _prep_core_inputs(embeds, weights, tpc, depth, n_cores)

    key = (tpc, depth)
    if key not in _NC_CACHE:
        _NC_CACHE[key] = build_nc(tpc=tpc, depth=depth)
    nc = _NC_CACHE[key]

    res = run_bass_kernel_spmd(nc, in_maps, core_ids=list(range(n_cores)),
                               trace=_trace)
    outs = np.concatenate([r["out"] for r in res.results], axis=0)
    if _trace:
        kernel.last_results = res
    return outs.astype(np.float32)


kernel.last_results = None


# revision 21
# speedup vs baseline: 1.0806x; 1.0081x over previous
"""BatchChildSumTreeLSTM Trainium2 kernel.

Forest of T complete B-ary trees, level-major node layout, processed
leaves-first.  Sharded across 8 NeuronCores by tree (2 trees / core);
every level's per-core slice is contiguous, so the whole recursion is
device-local.

Per-core layout is feature-major: activations live in SBUF as
[128 features (partitions), n nodes (free)].  Matmuls are
out[H, n] = W.T.T @ rhs with pre-transposed weights as lhsT, so matmul
outputs are directly consumable as later matmul inputs and PSUM
accumulates the x-projection with the h-recurrence.  The host feeds:
  xT  [128, ntot]  embeds transposed (per-core shard)
  xb  [128, ntot]  parent embedding broadcast to each child column
                   (level-0 columns unused) -> f-gate is 2 matmuls.
Gates are evaluated by ScalarE straight out of PSUM with per-partition
bias APs; VectorE does the elementwise muls and the fc child-sum
(pairwise tree); GpSimd does the h child-sum.  Level 6 h/c spill to
DRAM (too big for SBUF together with the level-7 streaming chunks);
levels <= 5 stay resident.
"""

import sys

if "/opt/trn_rl_repo" not in sys.path:
    sys.path.insert(0, "/opt/trn_rl_repo")

import numpy as np

P = 128          # feature dim == partitions
BR = 4           # branching factor
NLBL = 5

F32 = None       # filled lazily (mybir import is heavy)

_NC_CACHE = {}


def _levels(tpc, depth):
    n = [tpc * BR**l for l in range(depth)]
    off = [0]
    for c in n:
        off.append(off[-1] + c)
    return n, off, off[-1]


def _legalize_waits(nc, max_waits=1):
    """This walrus build accepts at most one sync-wait command per
    instruction (any type).  Hoist excess waits onto same-engine NoOps
    inserted right before the instruction; engine program order makes
    this exactly equivalent."""
    import concourse.mybir as mybir

    n_nops = 0
    for fn in nc.m.functions:
        for blk in fn.blocks:
            new_insts = []
            for inst in blk.instructions:
                si = getattr(inst, "sync_info", None)
                if si is not None and si.on_wait and len(si.on_wait) > max_waits:
                    waits = list(si.on_wait)
                    hoist, keep = waits[:-max_waits], waits[-max_waits:]
                    eng = getattr(inst, "engine", None)
                    for j, w in enumerate(hoist):
                        nop = mybir.InstNoOp(
                            name=f"{inst.name}-wn{j}",
                            engine=eng,
                            bass_nofuse=True,
                        )
                        nop.sync_info = mybir.SyncInfo(on_wait=[w],
                                                       on_update=[])
                        new_insts.append(nop)
                        n_nops += 1
                    inst.sync_info = mybir.SyncInfo(
                        on_wait=keep, on_update=list(si.on_update))
                new_insts.append(inst)
            blk.instructions = new_insts
    return n_nops


def build_nc(tpc=2, depth=8, ch_leaf=2048, resident_max=2048, nlbl=NLBL,
             legalize=True):
    """Build the per-core Bass/Tile program (identical on all cores)."""
    import concourse.bass as bass
    import concourse.mybir as mybir
    import concourse.tile as tile

    f32 = mybir.dt.float32
    bf16 = mybir.dt.bfloat16
    AF = mybir.ActivationFunctionType
    ADD = mybir.AluOpType.add

    n, off, ntot = _levels(tpc, depth)
    lleaf = depth - 1
    nleaf = n[lleaf]
    ch_leaf = min(ch_leaf, nleaf)
    assert nleaf % ch_leaf == 0

    nc = bass.Bass()

    xT = nc.dram_tensor("xT", [P, ntot], bf16, kind="ExternalInput")
    xb = nc.dram_tensor("xb", [P, ntot], bf16, kind="ExternalInput")
    wnames = ["ix", "ih", "fx", "fh", "ox", "oh", "ux", "uh"]
    wall = nc.dram_tensor("Wall", [P, len(wnames), P], bf16,
                          kind="ExternalInput")
    bias4 = nc.dram_tensor("bias4", [P, 4], f32, kind="ExternalInput")
    woutT = nc.dram_tensor("WoutT", [P, nlbl], bf16, kind="ExternalInput")
    bout = nc.dram_tensor("bout2", [tpc, nlbl], f32, kind="ExternalInput")
    out = nc.dram_tensor("out", [tpc, nlbl], f32, kind="ExternalOutput")

    assert depth >= 4

    SIG, TANH = AF.Sigmoid, AF.Tanh
    BIDX = {"i": 0, "f": 1, "o": 2, "u": 3}

    with tile.TileContext(nc) as tc:
        import contextlib
        with contextlib.ExitStack() as ctx:
            wp = ctx.enter_context(tc.tile_pool(name="wp", bufs=1))
            stream = ctx.enter_context(tc.tile_pool(name="stream", bufs=2))
            tmp = ctx.enter_context(tc.tile_pool(name="tmp", bufs=2))
            acc = ctx.enter_context(tc.tile_pool(name="acc", bufs=2))
            chk = ctx.enter_context(tc.tile_pool(name="chk", bufs=2))
            hcp = ctx.enter_context(tc.tile_pool(name="hcp", bufs=1))
            psum = ctx.enter_context(
                tc.tile_pool(name="psum", bufs=2, space="PSUM"))

            GMAX = 2048  # max columns per gate/f group (4 PSUM banks)

            # ---- load constants ----
            wall_sb = wp.tile([P, len(wnames), P], bf16, name="wall_sb",
                              tag="wall_sb")
            nc.sync.dma_start(out=wall_sb, in_=wall[:])
            wsb = {nm: wall_sb[:, j, :] for j, nm in enumerate(wnames)}
            bias_sb = wp.tile([P, 4], f32, name="bias_sb", tag="bias_sb")
            nc.sync.dma_start(out=bias_sb, in_=bias4[:])
            woutT_sb = wp.tile([P, nlbl], bf16, name="woutT_sb", tag="woutT_sb")
            nc.sync.dma_start(out=woutT_sb, in_=woutT[:])
            bout_sb = wp.tile([tpc, nlbl], f32, name="bout_sb", tag="bout_sb")
            nc.sync.dma_start(out=bout_sb, in_=bout[:])

            def bias_ap(g):
                i = BIDX[g]
                return bias_sb[:, i:i + 1]

            # persistent h/c for resident levels (leaves and level
            # depth-2 are streamed/consumed in flight, never stored whole)
            hres, cres = {}, {}
            for l in range(depth - 2):
                hres[l] = hcp.tile([P, n[l]], bf16, name=f"h{l}_sb",
                                   tag=f"h{l}_sb")
                cres[l] = hcp.tile([P, n[l]], f32, name=f"c{l}_sb",
                                   tag=f"c{l}_sb")

            def matmul_group(ps, w0, rhs0, w1=None, rhs1=None, G=GMAX):
                """ps[:, :G] = w0.T@rhs0 (+ w1.T@rhs1).  512-col banks."""
                nb = (G + 511) // 512
                for b in range(nb):
                    s = b * 512
                    e = min(s + 512, G)
                    nc.tensor.matmul(ps[:, s:e], wsb[w0], rhs0[:, s:e],
                                     start=True, stop=(w1 is None))
                if w1 is not None:
                    for b in range(nb):
                        s = b * 512
                        e = min(s + 512, G)
                        nc.tensor.matmul(ps[:, s:e], wsb[w1], rhs1[:, s:e],
                                         start=False, stop=True)

            def gate_pass(xt_ap, hs_ap, fcs_ap, h_out, c_out, G):
                """Compute i,u,o gates + c,h for G parent columns."""
                leaf = hs_ap is None
                g_sb = {}
                for gname, wx, wh, func in (("i", "ix", "ih", SIG),
                                            ("u", "ux", "uh", TANH),
                                            ("o", "ox", "oh", SIG)):
                    ps = psum.tile([P, GMAX], f32, name=f"ps_{gname}",
                                   tag="ps")
                    if leaf:
                        matmul_group(ps, wx, xt_ap, G=G)
                    else:
                        matmul_group(ps, wx, xt_ap, wh, hs_ap, G=G)
                    gdt = bf16 if gname == "o" else f32
                    g = tmp.tile([P, G], gdt, name=f"g_{gname}",
                                 tag=f"g_{gname}")
                    nc.scalar.activation(out=g, in_=ps[:, :G], func=func,
                                         bias=bias_ap(gname), scale=1.0)
                    g_sb[gname] = g
                nc.vector.tensor_mul(c_out, g_sb["i"], g_sb["u"])
                if fcs_ap is not None:
                    nc.vector.tensor_add(c_out, c_out, fcs_ap)
                tt = tmp.tile([P, G], bf16, name="tt", tag="tt")
                nc.scalar.activation(out=tt, in_=c_out, func=TANH,
                                     bias=0.0, scale=1.0)
                nc.vector.tensor_mul(h_out, g_sb["o"], tt)

            def f_pass(xb_ap, hch_ap, cch_ap, fcs_out, hs_out, S):
                """f gates for S child cols; reduce fc and h by groups of 4."""
                ps = psum.tile([P, GMAX], f32, name="ps_f", tag="ps")
                matmul_group(ps, "fx", xb_ap, "fh", hch_ap, G=S)
                f = tmp.tile([P, S], f32, name="g_f", tag="g_f", bufs=1)
                nc.scalar.activation(out=f, in_=ps[:, :S], func=SIG,
                                     bias=bias_ap("f"), scale=1.0)
                fc = tmp.tile([P, S], f32, name="fc", tag="fc", bufs=1)
                nc.vector.tensor_mul(fc, f, cch_ap)
                # pairwise reduce x4 -> fcs_out  (VectorE)
                fcv = fc.rearrange("p (a two) -> p a two", two=2)
                rt = tmp.tile([P, S // 2], f32, name="rt", tag="rt",
                              bufs=1)
                nc.vector.tensor_add(rt, fcv[:, :, 0], fcv[:, :, 1])
                rtv = rt.rearrange("p (a two) -> p a two", two=2)
                nc.vector.tensor_add(fcs_out, rtv[:, :, 0], rtv[:, :, 1])
                # pairwise reduce h x4 -> hs_out  (GpSimd)
                hv = hch_ap.rearrange("p (a two) -> p a two", two=2)
                rt2 = tmp.tile([P, S // 2], bf16, name="rt2", tag="rt2",
                               bufs=1)
                nc.gpsimd.tensor_tensor(rt2, hv[:, :, 0], hv[:, :, 1], ADD)
                rt2v = rt2.rearrange("p (a two) -> p a two", two=2)
                nc.gpsimd.tensor_tensor(hs_out, rt2v[:, :, 0], rt2v[:, :, 1],
                                        ADD)

            _stream_bufs = {"xt": 3, "xb": 4}

            def load_stream(tag, src, a, b, dt=bf16, bufs=None):
                t = stream.tile([P, b - a], dt, name=tag, tag=tag,
                                bufs=_stream_bufs.get(tag, 2))
                nc.sync.dma_start(out=t, in_=src[:, a:b])
                return t

            # ================= leaves fused with level depth-2 ============
            l6 = depth - 2
            GQ = min(GMAX, n[l6])          # parents per level-l6 gate group
            cpq = max(1, (GQ * BR) // ch_leaf)   # leaf chunks per gate group
            assert GQ * BR == cpq * ch_leaf
            nchunks = nleaf // ch_leaf
            fcs = hs = None
            pending = None
            l5 = depth - 3
            l4 = depth - 4
            q5 = GQ // BR
            q4 = max(1, q5 // BR)
            # level depth-3 / depth-4 child-sum accumulators
            fcs5 = acc.tile([P, n[l5]], f32, name="fcs5", tag="fcs5", bufs=1)
            hs5 = acc.tile([P, n[l5]], bf16, name="hs5", tag="hs5", bufs=1)
            fcs4 = acc.tile([P, n[l4]], f32, name="fcs4", tag="fcs4", bufs=1)
            hs4 = acc.tile([P, n[l4]], bf16, name="hs4", tag="hs4", bufs=1)

            def emit_l6_gates(xt6, hs_p, fcs_p, p0):
                h6o = chk.tile([P, GQ], bf16, name="h6o", tag="h6o")
                c6o = chk.tile([P, GQ], f32, name="c6o", tag="c6o")
                gate_pass(xt6, hs_p, fcs_p, h6o, c6o, GQ)
                return (h6o, c6o, p0)

            def emit_l5_f(h6o, c6o, p0):
                # f-pass of level depth-3 over the group produced above
                xbt5 = load_stream("xb", xb, off[l6] + p0, off[l6] + p0 + GQ)
                a5 = p0 // BR
                f_pass(xbt5, h6o, c6o, fcs5[:, a5:a5 + q5],
                       hs5[:, a5:a5 + q5], GQ)

            def emit_l5_gates(a5):
                xt5g = load_stream("xt", xT, off[l5] + a5, off[l5] + a5 + q5)
                gate_pass(xt5g, hs5[:, a5:a5 + q5], fcs5[:, a5:a5 + q5],
                          hres[l5][:, a5:a5 + q5], cres[l5][:, a5:a5 + q5],
                          q5)

            def emit_l4_f(a5):
                # children: level depth-3 cols [a5, a5+q5)
                xbt4 = load_stream("xb", xb, off[l5] + a5, off[l5] + a5 + q5)
                a4 = a5 // BR
                f_pass(xbt4, hres[l5][:, a5:a5 + q5], cres[l5][:, a5:a5 + q5],
                       fcs4[:, a4:a4 + q4], hs4[:, a4:a4 + q4], q5)

            pending_l5f = None
            pending_l5g = None
            pending_l4f = None
            pending_f = None
            q = ch_leaf // BR

            def emit_leaf_f(xbt_p, h7_p, c7_p, kp):
                # f-pass of level l6 over chunk kp's children (one chunk old,
                # so h/c are long since ready and PE never stalls on them)
                nonlocal fcs, hs, pending
                jq = kp % cpq
                if jq == 0:
                    fcs = acc.tile([P, GQ], f32, name="fcs", tag="fcs")
                    hs = acc.tile([P, GQ], bf16, name="hs", tag="hs")
                f_pass(xbt_p, h7_p, c7_p, fcs[:, jq * q:(jq + 1) * q],
                       hs[:, jq * q:(jq + 1) * q], ch_leaf)
                if jq == cpq - 1:
                    gq_i = kp // cpq
                    p0 = gq_i * GQ
                    xt6 = load_stream("xt", xT, off[l6] + p0,
                                      off[l6] + p0 + GQ)
                    pending = (xt6, hs, fcs, p0)

            def load_chunk_srcs(k):
                a = off[lleaf] + k * ch_leaf
                b = off[lleaf] + (k + 1) * ch_leaf
                return (load_stream("xt", xT, a, b),
                        load_stream("xb", xb, a, b))

            nxt = load_chunk_srcs(0)
            for k in range(nchunks):
                xt7, xbt = nxt
                if k + 1 < nchunks:
                    nxt = load_chunk_srcs(k + 1)  # prefetch next chunk's DMAs
                h7k = stream.tile([P, ch_leaf], bf16, name="hch", tag="hch")
                c7k = stream.tile([P, ch_leaf], f32, name="cch", tag="cch")
                gate_pass(xt7, None, None, h7k, c7k, ch_leaf)
                # deferred work from previous chunks/groups: ACT chews the
                # leaf sigmoids while PE runs these matmuls on old data
                if pending_l4f is not None:
                    emit_l4_f(pending_l4f)
                    pending_l4f = None
                if pending_l5g is not None:
                    emit_l5_gates(pending_l5g)
                    pending_l4f = pending_l5g
                    pending_l5g = None
                if pending_l5f is not None:
                    emit_l5_f(*pending_l5f)
                    pending_l5g = pending_l5f[2] // BR
                    pending_l5f = None
                if pending is not None:
                    pending_l5f = emit_l6_gates(*pending)
                    pending = None
                emit_leaf_f(xbt, h7k, c7k, k)
            # drain the software pipeline
            if pending is not None:
                pending_l5f = emit_l6_gates(*pending)
                pending = None
            if pending_l5f is not None:
                emit_l5_f(*pending_l5f)
                pending_l5g = pending_l5f[2] // BR
                pending_l5f = None
            if pending_l5g is not None:
                emit_l5_gates(pending_l5g)
                pending_l4f = pending_l5g
                pending_l5g = None
            if pending_l4f is not None:
                emit_l4_f(pending_l4f)
                pending_l4f = None

            # gates of level depth-4 (its child-sums are complete now)
            xt4 = load_stream("xt", xT, off[l4], off[l4] + n[l4])
            gate_pass(xt4, hs4, fcs4, hres[l4], cres[l4], n[l4])

            # ================= levels depth-5 .. 0 ========================
            for l in range(depth - 5, -1, -1):
                npar, nch = n[l], n[l + 1]
                S = min(GMAX, nch)
                ngr = nch // S
                fcs = acc.tile([P, npar], f32, name="fcs", tag="fcs")
                hs = acc.tile([P, npar], bf16, name="hs", tag="hs")
                for g in range(ngr):
                    a = g * S
                    hch = hres[l + 1][:, a:a + S]
                    cch = cres[l + 1][:, a:a + S]
                    xbt = load_stream("xb", xb, off[l + 1] + a,
                                      off[l + 1] + a + S)
                    q = S // BR
                    f_pass(xbt, hch, cch, fcs[:, g * q:(g + 1) * q],
                           hs[:, g * q:(g + 1) * q], S)
                xt = load_stream("xt", xT, off[l], off[l] + npar)
                gate_pass(xt, hs, fcs, hres[l], cres[l], npar)

            # ================= output head ================================
            ps = psum.tile([P, GMAX], f32, name="ps_out", tag="ps")
            nc.tensor.matmul(ps[:tpc, :nlbl], hres[0], woutT_sb,
                             start=True, stop=True)
            out_sb = tmp.tile([tpc, nlbl], f32, name="out_sb", tag="out_sb")
            nc.vector.tensor_add(out_sb, ps[:tpc, :nlbl], bout_sb)
            nc.sync.dma_start(out=out[:], in_=out_sb)

    if legalize:
        _legalize_waits(nc)
    return nc


def _prep_core_inputs(embeds, weights, tpc, depth, n_cores):
    """Host-side shard + transpose + parent-broadcast per core."""
    n, off, ntot = _levels(tpc, depth)
    T = tpc * n_cores
    counts = [T * BR**l for l in range(depth)]
    offsets = [0]
    for c in counts:
        offsets.append(offsets[-1] + c)

    common = dict(weights)
    in_maps = []
    for d in range(n_cores):
        import ml_dtypes
        bf16 = ml_dtypes.bfloat16
        shard = np.concatenate(
            [embeds[offsets[l] + tpc * d * BR**l:
                    offsets[l] + tpc * (d + 1) * BR**l] for l in range(depth)],
            axis=0)
        xT = np.ascontiguousarray(shard.T.astype(bf16))   # [P, ntot]
        xbm = np.zeros_like(xT)
        for l in range(1, depth):
            xbm[:, off[l]:off[l + 1]] = np.repeat(
                xT[:, off[l - 1]:off[l]], BR, axis=1)
        m = {"xT": xT, "xb": xbm}
        m.update(common)
        in_maps.append(m)
    return in_maps


def _prep_weights(Wix, bix, Wih, Wfx, bfx, Wfh, Wox, box, Woh, Wux, bux, Wuh,
                  Wout, bout, tpc):
    import ml_dtypes
    f = np.float32
    bf = ml_dtypes.bfloat16
    # order must match build_nc's wnames: ix, ih, fx, fh, ox, oh, ux, uh
    wall = np.stack([Wix.T, Wih.T, Wfx.T, Wfh.T, Wox.T, Woh.T, Wux.T, Wuh.T],
                    axis=1)   # [128 (in-feat), 8, 128 (out-feat)]
    w = {
        "Wall": np.ascontiguousarray(wall, dtype=bf),
        "bias4": np.ascontiguousarray(
            np.stack([bix, bfx, box, bux], axis=1), dtype=f),
        "WoutT": np.ascontiguousarray(Wout.T, dtype=bf),
        "bout2": np.ascontiguousarray(np.tile(bout, (tpc, 1)), dtype=f),
    }
    return w


def _ensure_ntff_hook():
    """The RL container's antenv lacks axon_hooks; install a shim and
    register the ctypes NTFF profiler so trace=True works."""
    import types

    try:
        from antenv.axon_hooks import get_axon_ntff_profile_hook  # noqa
        return
    except ImportError:
        pass
    mod = types.ModuleType("antenv.axon_hooks")
    _h = [None]
    mod.set_axon_ntff_profile_hook = lambda h: _h.__setitem__(0, h)
    mod.get_axon_ntff_profile_hook = lambda: _h[0]
    sys.modules["antenv.axon_hooks"] = mod
    import antenv
    antenv.axon_hooks = mod
    try:
        from trn_agent_boot.trn_boot import _ntff_profile_via_ctypes
        h = _ntff_profile_via_ctypes("/opt/axon/libaxon_pjrt.so")
        if h is not None:
            mod.set_axon_ntff_profile_hook(h)
    except Exception:
        pass


def kernel(embeds, Wix, bix, Wih, Wfx, bfx, Wfh, Wox, box, Woh, Wux, bux, Wuh,
           Wout, bout, _trace=False):
    from concourse import bass_utils
    from concourse.bass_utils import run_bass_kernel_spmd

    if _trace:
        _ensure_ntff_hook()
        bass_utils.upload_artifacts = lambda d: d  # no S3 in this container

    n_cores = 8
    depth = 8
    T = 16
    tpc = T // n_cores

    embeds = np.asarray(embeds, dtype=np.float32)
    weights = _prep_weights(
        np.asarray(Wix), np.asarray(bix), np.asarray(Wih), np.asarray(Wfx),
        np.asarray(bfx), np.asarray(Wfh), np.asarray(Wox), np.asarray(box),
        np.asarray(Woh), np.asarray(Wux), np.asarray(bux), np.asarray(Wuh),
        np.asarray(Wout), np.asarray(bout), tpc)
    in_maps = _prep_core_inputs(embeds, weights, tpc, depth, n_cores)

    key = (tpc, depth)
    if key not in _NC_CACHE:
        _NC_CACHE[key] = build_nc(tpc=tpc, depth=depth)
    nc = _NC_CACHE[key]

    res = run_bass_kernel_spmd(nc, in_maps, core_ids=list(range(n_cores)),
                               trace=_trace)
    outs = np.concatenate([r["out"] for r in res.results], axis=0)
    if _trace:
        kernel.last_results = res
    return outs.astype(np.float32)


kernel.last_results = None
